# revision 16
# baseline (speedup 1.0000x reference)
"""OIM loss with circular queue — Trainium2 Bass kernel (8 NeuronCores).

loss = mean_b [ M + log(sum_{q good} exp(30*cos(x_b,e_q) - M)) - 30*cos(x_b,e_{xe_b}) ]

where e is the circular queue after the (sequential, data-dependent) update.

Split of labor:
  host: the integer queue-update bookkeeping, the per-pid masked means
    (normalized exactly, fp8-quantized — they become ordinary queue columns),
    the target cosines t30_b (exact f64 dot with the normalized mean), and
    the final log/mean. The heavy B x Q x D cosine matmul and the B x Q
    exponentials run on the 8 cores.
  device (per core, tensor-parallel over Q): 2016 queue columns (32 window
    slots + up to 1984 good non-window slots, zero-padded). 32 b-tiles of
    fp8 DoubleRow matmuls fill two PSUM tiles (psmA [128,1024] + psmD
    [128,992] — separate tiles so their readers aren't serialized by the
    tile framework's same-tile reader chaining) with cosines; the
    exponentials are split across three engines:
      * ACT: native Exp (scale=30, bias=-M) on psmA with accum_out giving
        that range's row-sum directly.
      * DVE: Schraudolph exp on psmD — one fused mult+add tensor_scalar
        emitting int16 bf16-bit-patterns (exp(z) ~= bitcast_bf16(
        rint(z*128*log2e + 16256 + C)), C calibrated so the softmax-sum
        error is ~1e-4).
      * Pool (gpsimd): pairwise halving-add of the bf16 exps (SBUF-only
        engine), then DVE row-sums the halved row in 4x mode one tile
        later (so the Pool round-trip never stalls the DVE stream).
Host: S_b = sum_c (sA + sD)_c - n_zero*e^-M;  loss = mean(M + log S_b - t30_b).
"""

import os
import sys

import numpy as np

for _p in ("/opt/trn_rl_repo", "/root/.axon_site/_ro/trn_rl_repo"):
    if os.path.isdir(_p) and _p not in sys.path:
        sys.path.insert(0, _p)

B, D, Q, U = 4096, 512, 16384, 256
N_CORES = 8
UC = U // N_CORES           # 32 window slots per core
F_A = 1024                  # ACT exp cols (psmA, bank-aligned)
F_D = 992                   # DVE schraudolph cols (psmD)
QSC = F_A + F_D             # 2016 queue columns per core
NW = QSC - UC               # non-window columns per core (zero-padded)
HF = F_D // 2
MT = B // 128               # 32 b-tiles
MC = 4                      # b-tiles per xt DMA batch
KD = 2                      # matmul contraction chunks (DoubleRow pairs)
SUB = D // (128 * KD)       # 2
OIM_SCALAR = 30.0
M_BIAS = 30.0               # logits are <= 30 (both sides unit-norm)
IGNORE = -1
NPOOL_TAIL = 1              # last tiles skip Pool (shorter drain chain)
# tiles whose D-side exp runs on ACT (native Exp + accum) instead of the
# DVE schraudolph chain. Empty: handing ACT a tile turned out to LOSE time
# (the PE's in-order stream couples the A-side lag back into the D-side
# cadence), but the mechanism is kept for tuning.
SKIP_TILES = frozenset()

# Schraudolph-exp constants for exp(30*c - 30) emitted as bf16 bit patterns:
# i16 = rint(c*SCH_A + SCH_B); SCH_B holds the -30 bias, the bf16 exponent
# offset (127<<7) and the calibration constant C=-7.368 (zero weighted error
# over the cosine distribution of random unit vectors at D=512).
SCH_A = 5539.948957013619
SCH_B = 10708.683087674835

_PROG_CACHE = {}


def _build_program():
    import concourse.bacc as bacc
    import concourse.tile as tile
    from concourse import mybir

    f32 = mybir.dt.float32
    i16 = mybir.dt.int16
    bf16 = mybir.dt.bfloat16
    fp8 = mybir.dt.float8e4
    AF = mybir.ActivationFunctionType
    OP = mybir.AluOpType
    DR = mybir.MatmulPerfMode.DoubleRow

    nc = bacc.Bacc("TRN2", target_bir_lowering=False, debug=False,
                   num_devices=N_CORES)

    xt_d = nc.dram_tensor("xt", [B, D], fp8, kind="ExternalInput").ap()
    emb_d = nc.dram_tensor("emb", [128, (D // 128) * QSC], fp8,
                           kind="ExternalInput").ap()
    sacc_d = nc.dram_tensor("sacc", [128, 2 * MT], f32,
                            kind="ExternalOutput").ap()

    with tile.TileContext(nc) as tc:
        with (
            tc.tile_pool(name="singles", bufs=1) as singles,
            tc.tile_pool(name="twork", bufs=3) as twork,
            tc.tile_pool(name="ework", bufs=3) as ework,
        ):
            biasM = singles.tile([128, 1], f32)
            nc.vector.memset(biasM, -M_BIAS)
            # preload the Exp activation table while DMAs stream
            junk1 = singles.tile([128, 1], f32)
            nc.scalar.activation(out=junk1, in_=biasM, func=AF.Exp)

            sacc = singles.tile([128, 2 * MT], f32)

            # queue columns, all from DMA (window means host-computed).
            # embD first: DVE is the critical engine, so psmD's matmuls
            # must start earliest; embA last (ACT has slack to absorb it).
            # One DMA per region keeps runs >= 512B (no descriptor latency
            # penalty).
            embt = singles.tile([128, KD, SUB, QSC], fp8)
            emb4 = emb_d.rearrange("p (a b c) -> p a b c", a=KD, b=SUB)
            nc.sync.dma_start(out=embt[:, :, :, 0:F_D],
                              in_=emb4[:, :, :, 0:F_D])
            first_xt = twork.tile([128, 1, D], fp8, tag="tl1")
            nc.sync.dma_start(
                out=first_xt,
                in_=xt_d[0:128, :].rearrange("(j p) d -> p j d", j=1))
            # second xt batch (tiles 1-4) before embA: the D-side pipeline
            # (the critical DVE chain) must never wait behind embA's 1.5us
            # transfer; the delayed A-side absorbs embA's late arrival.
            second_xt = twork.tile([128, MC, D], fp8, tag=f"tl{MC}")
            nc.sync.dma_start(
                out=second_xt,
                in_=xt_d[128:128 * (1 + MC), :]
                .rearrange("(j p) d -> p j d", j=MC))
            nc.sync.dma_start(out=embt[:, :, :, F_D:QSC],
                              in_=emb4[:, :, :, F_D:QSC])

            # PE p-state warmup: the ramp (0.65 -> 2.4 GHz over 3us of
            # continuous busy) would otherwise burn the first ~5 tiles at
            # half speed. Dummy matmuls into the first psmA rotation buffer
            # keep the PE busy until the input DMAs land; the first real
            # matmul queues behind them with no idle gap, so the ramp
            # carries over.
            warm = singles.tile([128, 256], fp8)
            nc.gpsimd.memset(warm, 0)

            batches = [(0, 1)] + [(1 + k * MC, min(1 + (k + 1) * MC, MT))
                                  for k in range((MT - 1 + MC - 1) // MC)]
            pend = []           # (m, tile, ncols) rows awaiting the DVE sum

            def emit_sum(item):
                m_, t_, n_ = item
                sj = ework.tile([128, n_], bf16, tag="sj")
                nc.vector.tensor_scalar(
                    out=sj, in0=t_, scalar1=1.0, scalar2=None,
                    op0=OP.mult, op1=OP.add,
                    accum_out=sacc[:, MT + m_:MT + m_ + 1])

            with tc.tile_pool(name="psC", bufs=2, space="PSUM") as psC:
                pend_A = []     # (m, tlm) awaiting the delayed A side

                def emit_A(item):
                    m_, tlm_ = item
                    psmA = psC.tile([128, F_A], f32, tag="psmA")
                    for (p0, p1) in [(0, 512), (512, F_A)]:
                        for kd in range(KD):
                            nc.tensor.matmul(psmA[:, p0:p1], tlm_[:, kd],
                                             embt[:, kd, :,
                                                  F_D + p0:F_D + p1],
                                             start=(kd == 0),
                                             stop=(kd == KD - 1),
                                             perf_mode=DR)
                    scrA = ework.tile([128, F_A], bf16, tag="scrA")
                    nc.scalar.activation(out=scrA, in_=psmA,
                                         func=AF.Exp, bias=biasM,
                                         scale=OIM_SCALAR,
                                         accum_out=sacc[:, m_:m_ + 1])

                psW = psC.tile([128, F_A], f32, tag="psmA")
                for w in range(15):
                    nc.tensor.matmul(psW[:, 0:256], warm[:, 0:128],
                                     warm, start=True, stop=True)
                for (b0, b1) in batches:
                    nb = b1 - b0
                    if b0 == 0:
                        tl4 = first_xt
                    elif b0 == 1:
                        tl4 = second_xt
                    else:
                        tl4 = twork.tile([128, nb, D], fp8, tag=f"tl{nb}")
                        nc.sync.dma_start(
                            out=tl4,
                            in_=xt_d[b0 * 128:b1 * 128, :]
                            .rearrange("(j p) d -> p j d", j=nb))
                    for j in range(nb):
                        m = b0 + j
                        tlm = tl4[:, j].rearrange("p (a b c) -> p a b c",
                                                  a=KD, b=SUB)
                        psmD = psC.tile([128, F_D], f32, tag="psmD")
                        for (p0, p1) in [(0, 512), (512, F_D)]:
                            for kd in range(KD):
                                nc.tensor.matmul(psmD[:, p0:p1], tlm[:, kd],
                                                 embt[:, kd, :, p0:p1],
                                                 start=(kd == 0),
                                                 stop=(kd == KD - 1),
                                                 perf_mode=DR)
                        if m in SKIP_TILES:
                            # ACT takes this tile's D side entirely
                            scrD = ework.tile([128, F_D], bf16, tag="scrD")
                            nc.scalar.activation(
                                out=scrD, in_=psmD, func=AF.Exp, bias=biasM,
                                scale=OIM_SCALAR,
                                accum_out=sacc[:, MT + m:MT + m + 1])
                            pend_A.append((m, tlm))
                            if len(pend_A) > 1:
                                emit_A(pend_A.pop(0))
                            continue
                        # DVE: schraudolph exp of psmD as bf16 bit patterns
                        eDP = ework.tile([128, F_D], i16, tag="eDP")
                        nc.vector.tensor_scalar(out=eDP, in0=psmD,
                                                scalar1=SCH_A, scalar2=SCH_B,
                                                op0=OP.mult, op1=OP.add)
                        # A side (psmA matmuls + ACT exp) delayed one tile:
                        # the D-side pipeline never queues behind embA or
                        # the A matmuls, and ACT's slack absorbs the shift
                        pend_A.append((m, tlm))
                        if len(pend_A) > 1:
                            emit_A(pend_A.pop(0))
                        ebf = eDP.bitcast(bf16)
                        if m < MT - NPOOL_TAIL:
                            # Pool: halve by pairwise add; DVE sums halved
                            # row one tile later (Pool latency overlaps the
                            # next schraudolph instead of stalling DVE)
                            ph = ework.tile([128, HF], bf16, tag="ph")
                            nc.gpsimd.tensor_tensor(out=ph, in0=ebf[:, 0:HF],
                                                    in1=ebf[:, HF:],
                                                    op=OP.add)
                            pend.append((m, ph, HF))
                        else:
                            # drain tail: skip the Pool round-trip
                            pend.append((m, ebf, F_D))
                        if len(pend) > 1:
                            emit_sum(pend.pop(0))
                while pend_A:
                    emit_A(pend_A.pop(0))
                while pend:
                    emit_sum(pend.pop(0))

            nc.sync.dma_start(out=sacc_d, in_=sacc)

    nc.compile()
    return nc


def _host_bookkeeping(labels, label_cq, header_cq):
    """Mirror the reference's integer-only queue-update semantics."""
    labels = np.asarray(labels).astype(np.int64)
    lab = np.asarray(label_cq).astype(np.int64).copy()
    h0 = int(np.asarray(header_cq))

    uq = np.unique(labels)
    if uq.size < U:
        uniq = np.concatenate([uq, np.full(U - uq.size, uq.min(), np.int64)])
    else:
        uniq = uq[:U]

    emb_src = np.full(Q, -1, np.int64)   # >=0: window slot written by uniq u
    h = h0 % Q
    for u in range(U):
        y = uniq[u]
        m = lab == y
        i = int(np.argmax(m)) if m.any() else 0
        inval = bool(m.any()) and (i != h)
        emb_src[h] = u
        lab[h] = y
        if inval:
            lab[i] = IGNORE
        h = (h + 1) % Q

    good = lab != IGNORE
    goodidx = np.flatnonzero(good)
    gl = lab[goodidx]
    vals, first = np.unique(gl, return_index=True)
    pos = np.searchsorted(vals, labels)
    assert np.all(vals[np.clip(pos, 0, vals.size - 1)] == labels), \
        "batch label missing from queue"
    xe = goodidx[first[pos]]
    return uniq, emb_src, good, xe


def _prepare(inputs, labels, emb_cq, label_cq, header_cq):
    import ml_dtypes
    e_dt = ml_dtypes.float8_e4m3

    x = np.ascontiguousarray(np.asarray(inputs, np.float32))
    emb_cq = np.ascontiguousarray(np.asarray(emb_cq, np.float32))
    labels_i = np.asarray(labels).astype(np.int64)

    uniq, emb_src, good, xe = _host_bookkeeping(labels, label_cq, header_cq)

    # per-pid means over the batch (sorted-group reduceat), normalized exactly
    order = np.argsort(labels_i, kind="stable")
    ls = labels_i[order]
    starts = np.flatnonzero(np.r_[True, ls[1:] != ls[:-1]])
    vals = ls[starts]
    sums = np.add.reduceat(x[order].astype(np.float64), starts, axis=0)
    counts = np.diff(np.r_[starts, ls.size])[:, None]
    means = sums / counts
    means /= np.maximum(np.linalg.norm(means, axis=1, keepdims=True), 1e-12)

    # window columns in slot order; invalidated window slots become zeros
    h0 = int(np.asarray(header_cq)) % Q
    wslot = (h0 + np.arange(U)) % Q
    u_valid = good[wslot]
    widx_of_uniq = np.searchsorted(vals, uniq)
    win_emb = means[widx_of_uniq] * u_valid[:, None]          # [U, D] f64

    # exact target cosines on the host
    xn64 = x.astype(np.float64)
    xn64 /= np.maximum(np.linalg.norm(xn64, axis=1, keepdims=True), 1e-12)
    w_idx = emb_src[xe]                      # target window index, -1=extra
    tgt = np.empty((B, D), np.float64)
    winrows = w_idx >= 0
    tgt[winrows] = means[widx_of_uniq[np.clip(w_idx, 0, U - 1)][winrows]]
    if (~winrows).any():
        eb = emb_cq[xe[~winrows]].astype(np.float64)
        tgt[~winrows] = eb
    t30 = OIM_SCALAR * np.einsum("bd,bd->b", xn64, tgt)

    # d-major row-normalized fp8 x for the logits lhsT (per-row 1/|x| folded
    # into the quantization)
    xn = (xn64.astype(np.float32)).astype(e_dt)
    Y = xn.reshape(MT, 128, KD, SUB, 128)
    xt = np.ascontiguousarray(Y.transpose(0, 4, 2, 3, 1).reshape(B, D))

    # queue columns per core: 32 window means + good non-window slots.
    # device layout: cols [0:F_D] = psmD range, [F_D:QSC] = psmA range;
    # the window means go at the start of the psmA range (arbitrary).
    nonwin = np.flatnonzero(good & (emb_src < 0))
    assert nonwin.size <= N_CORES * NW, "queue overflow vs compiled shape"
    parts = np.array_split(nonwin, N_CORES)
    in_maps = []
    n_zero = int((~u_valid).sum())
    for c in range(N_CORES):
        cols = parts[c]
        n_zero += NW - cols.size
        E = np.zeros((QSC, D), np.float32)
        E[:F_D] = emb_cq[cols[:F_D]]
        E[F_D:F_D + UC] = win_emb[c * UC:(c + 1) * UC]
        rest = cols[F_D:]
        E[F_D + UC:F_D + UC + rest.size] = emb_cq[rest]
        Z = E.astype(e_dt).reshape(QSC, KD, SUB, 128)
        embp = np.ascontiguousarray(
            Z.transpose(3, 1, 2, 0).reshape(128, KD * SUB * QSC))
        in_maps.append({"xt": xt, "emb": embp})
    return in_maps, t30, n_zero


def _combine(res_list, t30, n_zero):
    S = np.zeros(B, np.float64)
    for r in res_list:
        sa = r["sacc"].astype(np.float64)
        S += sa[:, :MT].T.reshape(B) + sa[:, MT:].T.reshape(B)
    S -= n_zero * np.exp(-float(M_BIAS))
    loss = np.mean(M_BIAS + np.log(S) - t30)
    return np.array(loss, dtype=np.float32)


def kernel(inputs, labels, emb_cq, label_cq, age_cq, header_cq):
    from concourse.bass_utils import run_bass_kernel_spmd

    in_maps, t30, n_zero = _prepare(inputs, labels, emb_cq, label_cq,
                                    header_cq)

    if "prog" not in _PROG_CACHE:
        _PROG_CACHE["prog"] = _build_program()
    nc = _PROG_CACHE["prog"]

    res = run_bass_kernel_spmd(nc, in_maps, core_ids=list(range(N_CORES)))
    return _combine(res.results, t30, n_zero)


# revision 23
# speedup vs baseline: 1.0239x; 1.0239x over previous
"""OIM loss with circular queue — Trainium2 Bass kernel (8 NeuronCores).

loss = mean_b [ M + log(sum_{q good} exp(30*cos(x_b,e_q) - M)) - 30*cos(x_b,e_{xe_b}) ]

where e is the circular queue after the (sequential, data-dependent) update.

Split of labor:
  host: the integer queue-update bookkeeping, the per-pid masked means
    (normalized exactly, fp8-quantized — they become ordinary queue columns),
    the target cosines t30_b (exact f64 dot with the normalized mean), and
    the final log/mean. The heavy B x Q x D cosine matmul and the B x Q
    exponentials run on the 8 cores.
  device (per core, tensor-parallel over Q): 2016 queue columns (32 window
    slots + up to 1984 good non-window slots, zero-padded). 32 b-tiles of
    fp8 DoubleRow matmuls fill two PSUM tiles (psmA [128,1024] + psmD
    [128,992] — separate tiles so their readers aren't serialized by the
    tile framework's same-tile reader chaining) with cosines; the
    exponentials are split across three engines:
      * ACT: native Exp (scale=30, bias=-M) on psmA with accum_out giving
        that range's row-sum directly.
      * DVE: Schraudolph exp on psmD — one fused mult+add tensor_scalar
        emitting int16 bf16-bit-patterns (exp(z) ~= bitcast_bf16(
        rint(z*128*log2e + 16256 + C)), C calibrated so the softmax-sum
        error is ~1e-4).
      * Pool (gpsimd): pairwise halving-add of the bf16 exps (SBUF-only
        engine), then DVE row-sums the halved row in 4x mode one tile
        later (so the Pool round-trip never stalls the DVE stream).
Host: S_b = sum_c (sA + sD)_c - n_zero*e^-M;  loss = mean(M + log S_b - t30_b).
"""

import os
import sys

import numpy as np

for _p in ("/opt/trn_rl_repo", "/root/.axon_site/_ro/trn_rl_repo"):
    if os.path.isdir(_p) and _p not in sys.path:
        sys.path.insert(0, _p)

B, D, Q, U = 4096, 512, 16384, 256
N_CORES = 8
UC = U // N_CORES           # 32 window slots per core
F_A = 1024                  # ACT exp cols (psmA, bank-aligned)
F_D = 992                   # DVE schraudolph cols (psmD)
QSC = F_A + F_D             # 2016 queue columns per core
NW = QSC - UC               # non-window columns per core (zero-padded)
HF = F_D // 2
MT = B // 128               # 32 b-tiles
MC = 4                      # b-tiles per xt DMA batch
KD = 2                      # matmul contraction chunks (DoubleRow pairs)
SUB = D // (128 * KD)       # 2
OIM_SCALAR = 30.0
M_BIAS = 30.0               # logits are <= 30 (both sides unit-norm)
IGNORE = -1
NPOOL_TAIL = 2              # last tiles skip Pool (shorter drain chain)
# tiles whose D-side exp runs on ACT (native Exp + accum) at the DRAIN
# instead of the DVE schraudolph chain. Mid-stream takeover loses time (the
# PE's in-order stream couples the A-side lag back into the D-side cadence),
# but for the last two tiles the psmD buffer can stay live until the end, so
# ACT absorbs the work after its own stream finishes, shortening the
# saturated DVE stream. Only tiles >= MT-2 are legal (PSUM buffer liveness).
SKIP_TILES = frozenset({31})

# Schraudolph-exp constants for exp(30*c - 30) emitted as bf16 bit patterns:
# i16 = rint(c*SCH_A + SCH_B); SCH_B holds the -30 bias, the bf16 exponent
# offset (127<<7) and the calibration constant C=-7.368 (zero weighted error
# over the cosine distribution of random unit vectors at D=512).
SCH_A = 5539.948957013619
SCH_B = 10708.683087674835

_PROG_CACHE = {}


def _build_program():
    import concourse.bacc as bacc
    import concourse.tile as tile
    from concourse import mybir

    f32 = mybir.dt.float32
    i16 = mybir.dt.int16
    bf16 = mybir.dt.bfloat16
    fp8 = mybir.dt.float8e4
    AF = mybir.ActivationFunctionType
    OP = mybir.AluOpType
    DR = mybir.MatmulPerfMode.DoubleRow

    nc = bacc.Bacc("TRN2", target_bir_lowering=False, debug=False,
                   num_devices=N_CORES)

    xt_d = nc.dram_tensor("xt", [B, D], fp8, kind="ExternalInput").ap()
    emb_d = nc.dram_tensor("emb", [128, (D // 128) * QSC], fp8,
                           kind="ExternalInput").ap()
    sacc_d = nc.dram_tensor("sacc", [128, 2 * MT], f32,
                            kind="ExternalOutput").ap()

    with tile.TileContext(nc) as tc:
        with (
            tc.tile_pool(name="singles", bufs=1) as singles,
            tc.tile_pool(name="twork", bufs=3) as twork,
            tc.tile_pool(name="ework", bufs=3) as ework,
        ):
            biasM = singles.tile([128, 1], f32)
            nc.vector.memset(biasM, -M_BIAS)
            # preload the Exp activation table while DMAs stream
            junk1 = singles.tile([128, 1], f32)
            nc.scalar.activation(out=junk1, in_=biasM, func=AF.Exp)

            sacc = singles.tile([128, 2 * MT], f32)

            # queue columns, all from DMA (window means host-computed).
            # embD first: DVE is the critical engine, so psmD's matmuls
            # must start earliest; embA last (ACT has slack to absorb it).
            # One DMA per region keeps runs >= 512B (no descriptor latency
            # penalty).
            embt = singles.tile([128, KD, SUB, QSC], fp8)
            emb4 = emb_d.rearrange("p (a b c) -> p a b c", a=KD, b=SUB)
            nc.sync.dma_start(out=embt[:, :, :, 0:F_D],
                              in_=emb4[:, :, :, 0:F_D])
            first_xt = twork.tile([128, 1, D], fp8, tag="tl1")
            nc.sync.dma_start(
                out=first_xt,
                in_=xt_d[0:128, :].rearrange("(j p) d -> p j d", j=1))
            # xt tiles 1-2 next (they gate the DVE-critical D matmuls of the
            # first tiles), then embA (gates only the slack-tolerant A side,
            # but ACT's cumulative finish time tracks its arrival 1:1), then
            # xt tiles 3-4.
            second_xt = twork.tile([128, MC, D], fp8, tag=f"tl{MC}")
            nc.sync.dma_start(
                out=second_xt[:, 0:1],
                in_=xt_d[128:128 * 2, :].rearrange("(j p) d -> p j d", j=1))
            nc.sync.dma_start(out=embt[:, :, :, F_D:QSC],
                              in_=emb4[:, :, :, F_D:QSC])
            nc.sync.dma_start(
                out=second_xt[:, 1:MC],
                in_=xt_d[128 * 2:128 * (1 + MC), :]
                .rearrange("(j p) d -> p j d", j=MC - 1))

            # PE p-state warmup: the ramp (0.65 -> 2.4 GHz over 3us of
            # continuous busy) would otherwise burn the first ~5 tiles at
            # half speed. Dummy matmuls into the first psmA rotation buffer
            # keep the PE busy until the input DMAs land; the first real
            # matmul queues behind them with no idle gap, so the ramp
            # carries over.
            warm = singles.tile([128, 256], fp8)
            nc.gpsimd.memset(warm, 0)

            batches = [(0, 1)] + [(1 + k * MC, min(1 + (k + 1) * MC, MT))
                                  for k in range((MT - 1 + MC - 1) // MC)]
            pend = []           # (m, tile, ncols) rows awaiting the DVE sum

            def emit_sum(item):
                m_, t_, n_ = item
                sj = ework.tile([128, n_], bf16, tag="sj")
                nc.vector.tensor_scalar(
                    out=sj, in0=t_, scalar1=1.0, scalar2=None,
                    op0=OP.mult, op1=OP.add,
                    accum_out=sacc[:, MT + m_:MT + m_ + 1])

            with tc.tile_pool(name="psC", bufs=2, space="PSUM") as psC:
                pend_A = []     # (m, tlm) awaiting the delayed A side
                pend_D = []     # (m, psmD) for drain-time ACT takeover

                def emit_A(item):
                    m_, tlm_ = item
                    psmA = psC.tile([128, F_A], f32, tag="psmA")
                    for (p0, p1) in [(0, 512), (512, F_A)]:
                        for kd in range(KD):
                            nc.tensor.matmul(psmA[:, p0:p1], tlm_[:, kd],
                                             embt[:, kd, :,
                                                  F_D + p0:F_D + p1],
                                             start=(kd == 0),
                                             stop=(kd == KD - 1),
                                             perf_mode=DR)
                    scrA = ework.tile([128, F_A], bf16, tag="scrA")
                    nc.scalar.activation(out=scrA, in_=psmA,
                                         func=AF.Exp, bias=biasM,
                                         scale=OIM_SCALAR,
                                         accum_out=sacc[:, m_:m_ + 1])

                psW = psC.tile([128, F_A], f32, tag="psmA")
                for w in range(15):
                    nc.tensor.matmul(psW[:, 0:256], warm[:, 0:128],
                                     warm, start=True, stop=True)
                for (b0, b1) in batches:
                    nb = b1 - b0
                    if b0 == 0:
                        tl4 = first_xt
                    elif b0 == 1:
                        tl4 = second_xt
                    else:
                        tl4 = twork.tile([128, nb, D], fp8, tag=f"tl{nb}")
                        nc.sync.dma_start(
                            out=tl4,
                            in_=xt_d[b0 * 128:b1 * 128, :]
                            .rearrange("(j p) d -> p j d", j=nb))
                    for j in range(nb):
                        m = b0 + j
                        tlm = tl4[:, j].rearrange("p (a b c) -> p a b c",
                                                  a=KD, b=SUB)
                        psmD = psC.tile([128, F_D], f32, tag="psmD")
                        for (p0, p1) in [(0, 512), (512, F_D)]:
                            for kd in range(KD):
                                nc.tensor.matmul(psmD[:, p0:p1], tlm[:, kd],
                                                 embt[:, kd, :, p0:p1],
                                                 start=(kd == 0),
                                                 stop=(kd == KD - 1),
                                                 perf_mode=DR)
                        if m in SKIP_TILES:
                            # ACT takes this tile's D side, emitted at the
                            # drain; psmD stays live (no later writer)
                            assert m >= MT - 2
                            pend_D.append((m, psmD))
                            pend_A.append((m, tlm))
                            if len(pend_A) > 1:
                                emit_A(pend_A.pop(0))
                            continue
                        # DVE: schraudolph exp of psmD as bf16 bit patterns
                        eDP = ework.tile([128, F_D], i16, tag="eDP")
                        nc.vector.tensor_scalar(out=eDP, in0=psmD,
                                                scalar1=SCH_A, scalar2=SCH_B,
                                                op0=OP.mult, op1=OP.add)
                        # A side (psmA matmuls + ACT exp) delayed one tile:
                        # the D-side pipeline never queues behind embA or
                        # the A matmuls, and ACT's slack absorbs the shift
                        pend_A.append((m, tlm))
                        if len(pend_A) > 1:
                            emit_A(pend_A.pop(0))
                        ebf = eDP.bitcast(bf16)
                        if m < MT - NPOOL_TAIL:
                            # Pool: halve by pairwise add; DVE sums halved
                            # row one tile later (Pool latency overlaps the
                            # next schraudolph instead of stalling DVE)
                            ph = ework.tile([128, HF], bf16, tag="ph")
                            nc.gpsimd.tensor_tensor(out=ph, in0=ebf[:, 0:HF],
                                                    in1=ebf[:, HF:],
                                                    op=OP.add)
                            pend.append((m, ph, HF))
                        else:
                            # drain tail: skip the Pool round-trip
                            pend.append((m, ebf, F_D))
                        if len(pend) > 1:
                            emit_sum(pend.pop(0))
                for (md, psd) in pend_D:
                    scrD = ework.tile([128, F_D], bf16, tag="scrD")
                    nc.scalar.activation(
                        out=scrD, in_=psd, func=AF.Exp, bias=biasM,
                        scale=OIM_SCALAR,
                        accum_out=sacc[:, MT + md:MT + md + 1])
                while pend_A:
                    emit_A(pend_A.pop(0))
                while pend:
                    emit_sum(pend.pop(0))

            nc.sync.dma_start(out=sacc_d, in_=sacc)

    nc.compile()
    return nc


def _host_bookkeeping(labels, label_cq, header_cq):
    """Mirror the reference's integer-only queue-update semantics."""
    labels = np.asarray(labels).astype(np.int64)
    lab = np.asarray(label_cq).astype(np.int64).copy()
    h0 = int(np.asarray(header_cq))

    uq = np.unique(labels)
    if uq.size < U:
        uniq = np.concatenate([uq, np.full(U - uq.size, uq.min(), np.int64)])
    else:
        uniq = uq[:U]

    emb_src = np.full(Q, -1, np.int64)   # >=0: window slot written by uniq u
    h = h0 % Q
    for u in range(U):
        y = uniq[u]
        m = lab == y
        i = int(np.argmax(m)) if m.any() else 0
        inval = bool(m.any()) and (i != h)
        emb_src[h] = u
        lab[h] = y
        if inval:
            lab[i] = IGNORE
        h = (h + 1) % Q

    good = lab != IGNORE
    goodidx = np.flatnonzero(good)
    gl = lab[goodidx]
    vals, first = np.unique(gl, return_index=True)
    pos = np.searchsorted(vals, labels)
    assert np.all(vals[np.clip(pos, 0, vals.size - 1)] == labels), \
        "batch label missing from queue"
    xe = goodidx[first[pos]]
    return uniq, emb_src, good, xe


def _prepare(inputs, labels, emb_cq, label_cq, header_cq):
    import ml_dtypes
    e_dt = ml_dtypes.float8_e4m3

    x = np.ascontiguousarray(np.asarray(inputs, np.float32))
    emb_cq = np.ascontiguousarray(np.asarray(emb_cq, np.float32))
    labels_i = np.asarray(labels).astype(np.int64)

    uniq, emb_src, good, xe = _host_bookkeeping(labels, label_cq, header_cq)

    # per-pid means over the batch (sorted-group reduceat), normalized exactly
    order = np.argsort(labels_i, kind="stable")
    ls = labels_i[order]
    starts = np.flatnonzero(np.r_[True, ls[1:] != ls[:-1]])
    vals = ls[starts]
    sums = np.add.reduceat(x[order].astype(np.float64), starts, axis=0)
    counts = np.diff(np.r_[starts, ls.size])[:, None]
    means = sums / counts
    means /= np.maximum(np.linalg.norm(means, axis=1, keepdims=True), 1e-12)

    # window columns in slot order; invalidated window slots become zeros
    h0 = int(np.asarray(header_cq)) % Q
    wslot = (h0 + np.arange(U)) % Q
    u_valid = good[wslot]
    widx_of_uniq = np.searchsorted(vals, uniq)
    win_emb = means[widx_of_uniq] * u_valid[:, None]          # [U, D] f64

    # exact target cosines on the host
    xn64 = x.astype(np.float64)
    xn64 /= np.maximum(np.linalg.norm(xn64, axis=1, keepdims=True), 1e-12)
    w_idx = emb_src[xe]                      # target window index, -1=extra
    tgt = np.empty((B, D), np.float64)
    winrows = w_idx >= 0
    tgt[winrows] = means[widx_of_uniq[np.clip(w_idx, 0, U - 1)][winrows]]
    if (~winrows).any():
        eb = emb_cq[xe[~winrows]].astype(np.float64)
        tgt[~winrows] = eb
    t30 = OIM_SCALAR * np.einsum("bd,bd->b", xn64, tgt)

    # d-major row-normalized fp8 x for the logits lhsT (per-row 1/|x| folded
    # into the quantization)
    xn = (xn64.astype(np.float32)).astype(e_dt)
    Y = xn.reshape(MT, 128, KD, SUB, 128)
    xt = np.ascontiguousarray(Y.transpose(0, 4, 2, 3, 1).reshape(B, D))

    # queue columns per core: 32 window means + good non-window slots.
    # device layout: cols [0:F_D] = psmD range, [F_D:QSC] = psmA range;
    # the window means go at the start of the psmA range (arbitrary).
    nonwin = np.flatnonzero(good & (emb_src < 0))
    assert nonwin.size <= N_CORES * NW, "queue overflow vs compiled shape"
    parts = np.array_split(nonwin, N_CORES)
    in_maps = []
    n_zero = int((~u_valid).sum())
    for c in range(N_CORES):
        cols = parts[c]
        n_zero += NW - cols.size
        E = np.zeros((QSC, D), np.float32)
        E[:F_D] = emb_cq[cols[:F_D]]
        E[F_D:F_D + UC] = win_emb[c * UC:(c + 1) * UC]
        rest = cols[F_D:]
        E[F_D + UC:F_D + UC + rest.size] = emb_cq[rest]
        Z = E.astype(e_dt).reshape(QSC, KD, SUB, 128)
        embp = np.ascontiguousarray(
            Z.transpose(3, 1, 2, 0).reshape(128, KD * SUB * QSC))
        in_maps.append({"xt": xt, "emb": embp})
    return in_maps, t30, n_zero


def _combine(res_list, t30, n_zero):
    S = np.zeros(B, np.float64)
    for r in res_list:
        sa = r["sacc"].astype(np.float64)
        S += sa[:, :MT].T.reshape(B) + sa[:, MT:].T.reshape(B)
    S -= n_zero * np.exp(-float(M_BIAS))
    loss = np.mean(M_BIAS + np.log(S) - t30)
    return np.array(loss, dtype=np.float32)


def kernel(inputs, labels, emb_cq, label_cq, age_cq, header_cq):
    from concourse.bass_utils import run_bass_kernel_spmd

    in_maps, t30, n_zero = _prepare(inputs, labels, emb_cq, label_cq,
                                    header_cq)

    if "prog" not in _PROG_CACHE:
        _PROG_CACHE["prog"] = _build_program()
    nc = _PROG_CACHE["prog"]

    res = run_bass_kernel_spmd(nc, in_maps, core_ids=list(range(N_CORES)))
    return _combine(res.results, t30, n_zero)


# revision 24
# speedup vs baseline: 1.0276x; 1.0036x over previous
"""OIM loss with circular queue — Trainium2 Bass kernel (8 NeuronCores).

loss = mean_b [ M + log(sum_{q good} exp(30*cos(x_b,e_q) - M)) - 30*cos(x_b,e_{xe_b}) ]

where e is the circular queue after the (sequential, data-dependent) update.

Split of labor:
  host: the integer queue-update bookkeeping, the per-pid masked means
    (normalized exactly, fp8-quantized — they become ordinary queue columns),
    the target cosines t30_b (exact f64 dot with the normalized mean), and
    the final log/mean. The heavy B x Q x D cosine matmul and the B x Q
    exponentials run on the 8 cores.
  device (per core, tensor-parallel over Q): 2016 queue columns (32 window
    slots + up to 1984 good non-window slots, zero-padded). 32 b-tiles of
    fp8 DoubleRow matmuls fill two PSUM tiles (psmA [128,1024] + psmD
    [128,992] — separate tiles so their readers aren't serialized by the
    tile framework's same-tile reader chaining) with cosines; the
    exponentials are split across three engines:
      * ACT: native Exp (scale=30, bias=-M) on psmA with accum_out giving
        that range's row-sum directly.
      * DVE: Schraudolph exp on psmD — one fused mult+add tensor_scalar
        emitting int16 bf16-bit-patterns (exp(z) ~= bitcast_bf16(
        rint(z*128*log2e + 16256 + C)), C calibrated so the softmax-sum
        error is ~1e-4).
      * Pool (gpsimd): pairwise halving-add of the bf16 exps (SBUF-only
        engine), then DVE row-sums the halved row in 4x mode one tile
        later (so the Pool round-trip never stalls the DVE stream).
Host: S_b = sum_c (sA + sD)_c - n_zero*e^-M;  loss = mean(M + log S_b - t30_b).
"""

import os
import sys

import numpy as np

for _p in ("/opt/trn_rl_repo", "/root/.axon_site/_ro/trn_rl_repo"):
    if os.path.isdir(_p) and _p not in sys.path:
        sys.path.insert(0, _p)

B, D, Q, U = 4096, 512, 16384, 256
N_CORES = 8
UC = U // N_CORES           # 32 window slots per core
F_A = 1024                  # ACT exp cols (psmA, bank-aligned)
F_D = 992                   # DVE schraudolph cols (psmD)
QSC = F_A + F_D             # 2016 queue columns per core
NW = QSC - UC               # non-window columns per core (zero-padded)
HF = F_D // 2
MT = B // 128               # 32 b-tiles
MC = 4                      # b-tiles per xt DMA batch
KD = 2                      # matmul contraction chunks (DoubleRow pairs)
SUB = D // (128 * KD)       # 2
OIM_SCALAR = 30.0
M_BIAS = 30.0               # logits are <= 30 (both sides unit-norm)
IGNORE = -1
NPOOL_TAIL = 2              # last tiles skip Pool (shorter drain chain)
# tiles whose D-side exp runs on ACT (native Exp + accum) at the DRAIN
# instead of the DVE schraudolph chain. Mid-stream takeover loses time (the
# PE's in-order stream couples the A-side lag back into the D-side cadence),
# but for the last two tiles the psmD buffer can stay live until the end, so
# ACT absorbs the work after its own stream finishes, shortening the
# saturated DVE stream. Only tiles >= MT-2 are legal (PSUM buffer liveness).
SKIP_TILES = frozenset({31})

# Schraudolph-exp constants for exp(30*c - 30) emitted as bf16 bit patterns:
# i16 = rint(c*SCH_A + SCH_B); SCH_B holds the -30 bias, the bf16 exponent
# offset (127<<7) and the calibration constant C=-7.368 (zero weighted error
# over the cosine distribution of random unit vectors at D=512).
SCH_A = 5539.948957013619
SCH_B = 10708.683087674835

_PROG_CACHE = {}


def _build_program():
    import concourse.bacc as bacc
    import concourse.tile as tile
    from concourse import mybir

    f32 = mybir.dt.float32
    i16 = mybir.dt.int16
    bf16 = mybir.dt.bfloat16
    fp8 = mybir.dt.float8e4
    AF = mybir.ActivationFunctionType
    OP = mybir.AluOpType
    DR = mybir.MatmulPerfMode.DoubleRow

    nc = bacc.Bacc("TRN2", target_bir_lowering=False, debug=False,
                   num_devices=N_CORES)

    xt_d = nc.dram_tensor("xt", [B, D], fp8, kind="ExternalInput").ap()
    emb_d = nc.dram_tensor("emb", [128, (D // 128) * QSC], fp8,
                           kind="ExternalInput").ap()
    sacc_d = nc.dram_tensor("sacc", [128, 2 * MT], f32,
                            kind="ExternalOutput").ap()

    with tile.TileContext(nc) as tc:
        with (
            tc.tile_pool(name="singles", bufs=1) as singles,
            tc.tile_pool(name="twork", bufs=3) as twork,
            tc.tile_pool(name="ework", bufs=3) as ework,
        ):
            biasM = singles.tile([128, 1], f32)
            nc.vector.memset(biasM, -M_BIAS)
            # preload the Exp activation table while DMAs stream
            junk1 = singles.tile([128, 1], f32)
            nc.scalar.activation(out=junk1, in_=biasM, func=AF.Exp)

            sacc = singles.tile([128, 2 * MT], f32)

            # queue columns, all from DMA (window means host-computed).
            # embD first: DVE is the critical engine, so psmD's matmuls
            # must start earliest; embA last (ACT has slack to absorb it).
            # One DMA per region keeps runs >= 512B (no descriptor latency
            # penalty).
            embt = singles.tile([128, KD, SUB, QSC], fp8)
            emb4 = emb_d.rearrange("p (a b c) -> p a b c", a=KD, b=SUB)
            nc.sync.dma_start(out=embt[:, :, :, 0:F_D],
                              in_=emb4[:, :, :, 0:F_D])
            first_xt = twork.tile([128, 1, D], fp8, tag="tl1")
            nc.sync.dma_start(
                out=first_xt,
                in_=xt_d[0:128, :].rearrange("(j p) d -> p j d", j=1))
            # xt tiles 1-2 next (they gate the DVE-critical D matmuls of the
            # first tiles), then embA (gates only the slack-tolerant A side,
            # but ACT's cumulative finish time tracks its arrival 1:1), then
            # xt tiles 3-4.
            second_xt = twork.tile([128, MC, D], fp8, tag=f"tl{MC}")
            nc.sync.dma_start(
                out=second_xt[:, 0:1],
                in_=xt_d[128:128 * 2, :].rearrange("(j p) d -> p j d", j=1))
            nc.sync.dma_start(out=embt[:, :, :, F_D:QSC],
                              in_=emb4[:, :, :, F_D:QSC])
            nc.sync.dma_start(
                out=second_xt[:, 1:MC],
                in_=xt_d[128 * 2:128 * (1 + MC), :]
                .rearrange("(j p) d -> p j d", j=MC - 1))

            # PE p-state warmup: the ramp (0.65 -> 2.4 GHz over 3us of
            # continuous busy) would otherwise burn the first ~5 tiles at
            # half speed. Dummy matmuls into the first psmA rotation buffer
            # keep the PE busy until the input DMAs land; the first real
            # matmul queues behind them with no idle gap, so the ramp
            # carries over.
            warm = singles.tile([128, 256], fp8)
            nc.gpsimd.memset(warm, 0)

            batches = [(0, 1)] + [(1 + k * MC, min(1 + (k + 1) * MC, MT))
                                  for k in range((MT - 1 + MC - 1) // MC)]
            pend = []           # (m, tile, ncols) rows awaiting the DVE sum

            def emit_sum(item):
                m_, t_, n_ = item
                sj = ework.tile([128, n_], bf16, tag="sj")
                nc.vector.tensor_scalar(
                    out=sj, in0=t_, scalar1=1.0, scalar2=None,
                    op0=OP.mult, op1=OP.add,
                    accum_out=sacc[:, MT + m_:MT + m_ + 1])

            with tc.tile_pool(name="psC", bufs=2, space="PSUM") as psC:
                pend_A = []     # (m, tlm) awaiting the delayed A side
                pend_D = []     # (m, psmD) for drain-time ACT takeover

                def emit_A(item):
                    m_, tlm_ = item
                    psmA = psC.tile([128, F_A], f32, tag="psmA")
                    for (p0, p1) in [(0, 512), (512, F_A)]:
                        for kd in range(KD):
                            nc.tensor.matmul(psmA[:, p0:p1], tlm_[:, kd],
                                             embt[:, kd, :,
                                                  F_D + p0:F_D + p1],
                                             start=(kd == 0),
                                             stop=(kd == KD - 1),
                                             perf_mode=DR)
                    scrA = ework.tile([128, F_A], bf16, tag="scrA")
                    nc.scalar.activation(out=scrA, in_=psmA,
                                         func=AF.Exp, bias=biasM,
                                         scale=OIM_SCALAR,
                                         accum_out=sacc[:, m_:m_ + 1])

                psW = psC.tile([128, F_A], f32, tag="psmA")
                for w in range(15):
                    nc.tensor.matmul(psW[:, 0:256], warm[:, 0:128],
                                     warm, start=True, stop=True)
                for (b0, b1) in batches:
                    nb = b1 - b0
                    if b0 == 0:
                        tl4 = first_xt
                    elif b0 == 1:
                        tl4 = second_xt
                    else:
                        tl4 = twork.tile([128, nb, D], fp8, tag=f"tl{nb}")
                        nc.sync.dma_start(
                            out=tl4,
                            in_=xt_d[b0 * 128:b1 * 128, :]
                            .rearrange("(j p) d -> p j d", j=nb))
                    for j in range(nb):
                        m = b0 + j
                        tlm = tl4[:, j].rearrange("p (a b c) -> p a b c",
                                                  a=KD, b=SUB)
                        psmD = psC.tile([128, F_D], f32, tag="psmD")
                        for (p0, p1) in [(0, 512), (512, F_D)]:
                            for kd in range(KD):
                                nc.tensor.matmul(psmD[:, p0:p1], tlm[:, kd],
                                                 embt[:, kd, :, p0:p1],
                                                 start=(kd == 0),
                                                 stop=(kd == KD - 1),
                                                 perf_mode=DR)
                        if m in SKIP_TILES:
                            # ACT takes this tile's D side, emitted at the
                            # drain; psmD stays live (no later writer)
                            assert m >= MT - 2
                            pend_D.append((m, psmD))
                            pend_A.append((m, tlm))
                            if len(pend_A) > 1:
                                emit_A(pend_A.pop(0))
                            continue
                        # DVE: schraudolph exp of psmD as bf16 bit patterns
                        eDP = ework.tile([128, F_D], i16, tag="eDP")
                        nc.vector.tensor_scalar(out=eDP, in0=psmD,
                                                scalar1=SCH_A, scalar2=SCH_B,
                                                op0=OP.mult, op1=OP.add)
                        # A side (psmA matmuls + ACT exp) delayed one tile:
                        # the D-side pipeline never queues behind embA or
                        # the A matmuls, and ACT's slack absorbs the shift
                        pend_A.append((m, tlm))
                        if len(pend_A) > 1:
                            emit_A(pend_A.pop(0))
                        ebf = eDP.bitcast(bf16)
                        if m < MT - NPOOL_TAIL:
                            # Pool: halve by pairwise add; DVE sums halved
                            # row one tile later (Pool latency overlaps the
                            # next schraudolph instead of stalling DVE)
                            ph = ework.tile([128, HF], bf16, tag="ph")
                            nc.gpsimd.tensor_tensor(out=ph, in0=ebf[:, 0:HF],
                                                    in1=ebf[:, HF:],
                                                    op=OP.add)
                            pend.append((m, ph, HF))
                        else:
                            # drain tail: skip the Pool round-trip
                            pend.append((m, ebf, F_D))
                        if len(pend) > 2:
                            emit_sum(pend.pop(0))
                for (md, psd) in pend_D:
                    scrD = ework.tile([128, F_D], bf16, tag="scrD")
                    nc.scalar.activation(
                        out=scrD, in_=psd, func=AF.Exp, bias=biasM,
                        scale=OIM_SCALAR,
                        accum_out=sacc[:, MT + md:MT + md + 1])
                while pend_A:
                    emit_A(pend_A.pop(0))
                while pend:
                    emit_sum(pend.pop(0))

            nc.sync.dma_start(out=sacc_d, in_=sacc)

    nc.compile()
    return nc


def _host_bookkeeping(labels, label_cq, header_cq):
    """Mirror the reference's integer-only queue-update semantics."""
    labels = np.asarray(labels).astype(np.int64)
    lab = np.asarray(label_cq).astype(np.int64).copy()
    h0 = int(np.asarray(header_cq))

    uq = np.unique(labels)
    if uq.size < U:
        uniq = np.concatenate([uq, np.full(U - uq.size, uq.min(), np.int64)])
    else:
        uniq = uq[:U]

    emb_src = np.full(Q, -1, np.int64)   # >=0: window slot written by uniq u
    h = h0 % Q
    for u in range(U):
        y = uniq[u]
        m = lab == y
        i = int(np.argmax(m)) if m.any() else 0
        inval = bool(m.any()) and (i != h)
        emb_src[h] = u
        lab[h] = y
        if inval:
            lab[i] = IGNORE
        h = (h + 1) % Q

    good = lab != IGNORE
    goodidx = np.flatnonzero(good)
    gl = lab[goodidx]
    vals, first = np.unique(gl, return_index=True)
    pos = np.searchsorted(vals, labels)
    assert np.all(vals[np.clip(pos, 0, vals.size - 1)] == labels), \
        "batch label missing from queue"
    xe = goodidx[first[pos]]
    return uniq, emb_src, good, xe


def _prepare(inputs, labels, emb_cq, label_cq, header_cq):
    import ml_dtypes
    e_dt = ml_dtypes.float8_e4m3

    x = np.ascontiguousarray(np.asarray(inputs, np.float32))
    emb_cq = np.ascontiguousarray(np.asarray(emb_cq, np.float32))
    labels_i = np.asarray(labels).astype(np.int64)

    uniq, emb_src, good, xe = _host_bookkeeping(labels, label_cq, header_cq)

    # per-pid means over the batch (sorted-group reduceat), normalized exactly
    order = np.argsort(labels_i, kind="stable")
    ls = labels_i[order]
    starts = np.flatnonzero(np.r_[True, ls[1:] != ls[:-1]])
    vals = ls[starts]
    sums = np.add.reduceat(x[order].astype(np.float64), starts, axis=0)
    counts = np.diff(np.r_[starts, ls.size])[:, None]
    means = sums / counts
    means /= np.maximum(np.linalg.norm(means, axis=1, keepdims=True), 1e-12)

    # window columns in slot order; invalidated window slots become zeros
    h0 = int(np.asarray(header_cq)) % Q
    wslot = (h0 + np.arange(U)) % Q
    u_valid = good[wslot]
    widx_of_uniq = np.searchsorted(vals, uniq)
    win_emb = means[widx_of_uniq] * u_valid[:, None]          # [U, D] f64

    # exact target cosines on the host
    xn64 = x.astype(np.float64)
    xn64 /= np.maximum(np.linalg.norm(xn64, axis=1, keepdims=True), 1e-12)
    w_idx = emb_src[xe]                      # target window index, -1=extra
    tgt = np.empty((B, D), np.float64)
    winrows = w_idx >= 0
    tgt[winrows] = means[widx_of_uniq[np.clip(w_idx, 0, U - 1)][winrows]]
    if (~winrows).any():
        eb = emb_cq[xe[~winrows]].astype(np.float64)
        tgt[~winrows] = eb
    t30 = OIM_SCALAR * np.einsum("bd,bd->b", xn64, tgt)

    # d-major row-normalized fp8 x for the logits lhsT (per-row 1/|x| folded
    # into the quantization)
    xn = (xn64.astype(np.float32)).astype(e_dt)
    Y = xn.reshape(MT, 128, KD, SUB, 128)
    xt = np.ascontiguousarray(Y.transpose(0, 4, 2, 3, 1).reshape(B, D))

    # queue columns per core: 32 window means + good non-window slots.
    # device layout: cols [0:F_D] = psmD range, [F_D:QSC] = psmA range;
    # the window means go at the start of the psmA range (arbitrary).
    nonwin = np.flatnonzero(good & (emb_src < 0))
    assert nonwin.size <= N_CORES * NW, "queue overflow vs compiled shape"
    parts = np.array_split(nonwin, N_CORES)
    in_maps = []
    n_zero = int((~u_valid).sum())
    for c in range(N_CORES):
        cols = parts[c]
        n_zero += NW - cols.size
        E = np.zeros((QSC, D), np.float32)
        E[:F_D] = emb_cq[cols[:F_D]]
        E[F_D:F_D + UC] = win_emb[c * UC:(c + 1) * UC]
        rest = cols[F_D:]
        E[F_D + UC:F_D + UC + rest.size] = emb_cq[rest]
        Z = E.astype(e_dt).reshape(QSC, KD, SUB, 128)
        embp = np.ascontiguousarray(
            Z.transpose(3, 1, 2, 0).reshape(128, KD * SUB * QSC))
        in_maps.append({"xt": xt, "emb": embp})
    return in_maps, t30, n_zero


def _combine(res_list, t30, n_zero):
    S = np.zeros(B, np.float64)
    for r in res_list:
        sa = r["sacc"].astype(np.float64)
        S += sa[:, :MT].T.reshape(B) + sa[:, MT:].T.reshape(B)
    S -= n_zero * np.exp(-float(M_BIAS))
    loss = np.mean(M_BIAS + np.log(S) - t30)
    return np.array(loss, dtype=np.float32)


def kernel(inputs, labels, emb_cq, label_cq, age_cq, header_cq):
    from concourse.bass_utils import run_bass_kernel_spmd

    in_maps, t30, n_zero = _prepare(inputs, labels, emb_cq, label_cq,
                                    header_cq)

    if "prog" not in _PROG_CACHE:
        _PROG_CACHE["prog"] = _build_program()
    nc = _PROG_CACHE["prog"]

    res = run_bass_kernel_spmd(nc, in_maps, core_ids=list(range(N_CORES)))
    return _combine(res.results, t30, n_zero)
